# revision 21
# baseline (speedup 1.0000x reference)
"""CoverTreeLoss (hierarchical softmax CE) Trainium2 kernel.

Vocab-parallel over 8 NeuronCores: each core owns K/8 = 6250 classes.
Per core:
  phase A: indirect-DMA gather of each class's path rows (self row + up to
           16 ancestors, padded slots point at an appended zero row) from the
           full weights table in HBM, tree-reduce the 17 slots on DVE,
           transpose on the PE into awT [d=128, classes].
  phase B: logits chunk matmuls x_btT.T @ awT -> PSUM, copy to SBUF, DMA the
           logits shard out; per-chunk neg-max (DVE) + exp-sum (ACT accum)
           produce per-batch-row logsumexp partials.
Host combines the 8 shards: concat logits/added_weights, 8-way logsumexp
merge, mean CE loss.

Self-contained: hardcodes all shapes from the problem spec.
"""

import numpy as np

import concourse.bass as bass
import concourse.mybir as mybir
from concourse.bass_utils import run_bass_kernel_spmd
from concourse.masks import make_identity
from concourse.tile import TileContext

F32 = mybir.dt.float32
I32 = mybir.dt.int32

# Problem shapes (hardcoded per contract).
def configure(k=50000, length=65536, d=128, b=1024, dmax=16, ncores=8):
    global K, LENGTH, D, B, DMAX, NCORES, S, KSH, NT, KPAD, LEXT, ZROW, NBT, CH, NCH
    K, LENGTH, D, B, DMAX, NCORES = k, length, d, b, dmax, ncores
    S = DMAX + 1       # gather slots per class: self row + 16 path slots
    KSH = K // NCORES  # classes per core
    NT = (KSH + 127) // 128          # class tiles per core
    KPAD = NT * 128
    LEXT = LENGTH + 1                # weights + appended zero row
    ZROW = LENGTH                    # index of the zero row
    NBT = B // 128                   # batch tiles
    CH = 512                         # logits chunk (one PSUM bank of fp32)
    NCH = (KPAD + CH - 1) // CH
    _NC_CACHE.clear()


_NC_CACHE: dict = {}
configure()


# Instruction types whose walrus lowering accepts multiple sync waits.
# Compute ISA structs (matmul/TT/activation/...) carry exactly ONE sync-wait
# slot; extra waits must be hoisted onto preceding same-engine NoOps.
_MULTIWAIT_OK: set = set()


def _legalize_waits(nc: bass.Bass) -> None:
    """Compute/SWDGE ISA instructions carry exactly one sync-wait slot.

    Hoist all-but-one wait of any multi-wait instruction onto freshly
    created same-engine NoOps placed immediately before it. NoOps are
    created via the engine API (so CoreSim/bass bookkeeping sees them)
    and then moved into position.
    """
    plan = []  # (fn, blk, inst) with >1 waits
    for fn in nc.m.functions:
        for blk in fn.blocks:
            for inst in blk.instructions:
                si = inst.sync_info
                waits = list(si.on_wait) if si is not None and si.on_wait else []
                if len(waits) > 1 and type(inst).__name__ not in _MULTIWAIT_OK:
                    plan.append((blk, inst))
    for blk, inst in plan:
        si = inst.sync_info
        waits = list(si.on_wait)
        nops = []
        for wx in waits[:-1]:
            bn = nc.engines[inst.engine].nop()
            n = bn.ins
            n.sync_info = mybir.SyncInfo(on_wait=[wx], on_update=[])
            nops.append(n)
        inst.sync_info = mybir.SyncInfo(
            on_wait=[waits[-1]], on_update=list(si.on_update or [])
        )
        # remove the freshly appended nops from wherever add_instruction put
        # them, then splice before the target instruction
        nop_names = {n.name for n in nops}
        for fn2 in nc.m.functions:
            for b2 in fn2.blocks:
                if any(i.name in nop_names for i in b2.instructions):
                    b2.instructions = [
                        i for i in b2.instructions if i.name not in nop_names
                    ]
        insts = list(blk.instructions)
        pos = next(k for k, i in enumerate(insts) if i.name == inst.name)
        blk.instructions = insts[:pos] + nops + insts[pos:]


def build_nc(nslots) -> bass.Bass:
    nc = bass.Bass(target_bir_lowering=False)

    w = nc.dram_tensor("w", [LEXT, D], F32, kind="ExternalInput")
    xt = nc.dram_tensor("xt", [D, B], F32, kind="ExternalInput")
    idxT = nc.dram_tensor("idxT", [D, NT * S], I32, kind="ExternalInput")

    logits_sh = nc.dram_tensor("logits_sh", [B, KSH], F32, kind="ExternalOutput")
    aw_sh = nc.dram_tensor("aw_sh", [KSH, D], F32, kind="ExternalOutput")
    mx = nc.dram_tensor("mx", [128, NBT], F32, kind="ExternalOutput")
    sm = nc.dram_tensor("sm", [128, NBT], F32, kind="ExternalOutput")

    with TileContext(nc) as tc:
        with (
            tc.tile_pool(name="const", bufs=1) as constp,
            tc.tile_pool(name="gath", bufs=3) as gathp,
            tc.tile_pool(name="lout", bufs=3) as loutp,
            tc.tile_pool(name="scr", bufs=2) as scrp,
            tc.tile_pool(name="sml", bufs=4) as smlp,
            tc.tile_pool(name="pst", bufs=2, space="PSUM") as psumt,
            tc.tile_pool(name="psm", bufs=3, space="PSUM") as psummm,
        ):
            ident = constp.tile([128, 128], F32)
            make_identity(nc, ident[:])

            xt_sb = constp.tile([D, B], F32)
            nc.sync.dma_start(out=xt_sb[:], in_=xt[:])
            idx_sb = constp.tile([D, NT * S], I32)
            nc.sync.dma_start(out=idx_sb[:], in_=idxT[:])

            awT = constp.tile([D, KPAD], F32)       # [d, class] transposed
            nmx = constp.tile([128, NBT * NCH], F32)  # per-chunk -max
            ssm = constp.tile([128, NBT * NCH], F32)  # per-chunk sum(exp(.-max))
            mx_sb = constp.tile([128, NBT], F32)
            sm_sb = constp.tile([128, NBT], F32)

            # ---------------- Phase A: gather + segment-sum + transpose ----
            # One indirect DMA gathers 128 rows (one per partition). Classes
            # are pre-sorted by path length (host) so tile t only needs
            # nslots[t] = 1 + max path_len in the tile gather instructions.
            for t in range(NT):
                n = nslots[t]
                g = gathp.tile([128, S, D], F32, tag="g")
                for s in range(n):
                    nc.gpsimd.indirect_dma_start(
                        out=g[:, s, :],
                        out_offset=None,
                        in_=w[:],
                        in_offset=bass.IndirectOffsetOnAxis(
                            ap=idx_sb[:, t * S + s : t * S + s + 1], axis=0
                        ),
                    )
                # fold upper half into lower half until one slot remains
                m = n
                while m > 1:
                    h = m // 2
                    nc.vector.tensor_add(
                        out=g[:, 0:h, :],
                        in0=g[:, 0:h, :],
                        in1=g[:, m - h : m, :],
                    )
                    m -= h
                rows = min(128, KSH - t * 128)
                nc.sync.dma_start(
                    out=aw_sh[t * 128 : t * 128 + rows, :], in_=g[:rows, 0, :]
                )
                pt = psumt.tile([128, 128], F32, tag="pt")
                nc.tensor.transpose(out=pt[:], in_=g[:, 0, :], identity=ident[:])
                nc.scalar.copy(out=awT[:, t * 128 : (t + 1) * 128], in_=pt[:])

            # ---------------- Phase B: logits + softmax partials -----------
            for bt in range(NBT):
                for c in range(NCH):
                    c0 = c * CH
                    wf = min(CH, KPAD - c0)          # cols incl. padding
                    wv = min(KSH - c0, wf)           # valid (real-class) cols
                    pm = psummm.tile([128, CH], F32, tag="pm")
                    nc.tensor.matmul(
                        out=pm[:, :wf],
                        lhsT=xt_sb[:, bt * 128 : (bt + 1) * 128],
                        rhs=awT[:, c0 : c0 + wf],
                        start=True,
                        stop=True,
                    )
                    lc = loutp.tile([128, CH], F32, tag="lc")
                    nc.scalar.copy(out=lc[:, :wv], in_=pm[:, :wv])
                    i1 = bt * NCH + c
                    nc.vector.reduce_max(
                        out=nmx[:, i1 : i1 + 1],
                        in_=lc[:, :wv],
                        axis=mybir.AxisListType.X,
                        negate=True,
                    )
                    sc = scrp.tile([128, CH], F32, tag="sc")
                    nc.scalar.activation(
                        out=sc[:, :wv],
                        in_=lc[:, :wv],
                        func=mybir.ActivationFunctionType.Exp,
                        bias=nmx[:, i1 : i1 + 1],
                        scale=1.0,
                        accum_out=ssm[:, i1 : i1 + 1],
                    )
                    nc.sync.dma_start(
                        out=logits_sh[bt * 128 : (bt + 1) * 128, c0 : c0 + wv],
                        in_=lc[:, :wv],
                    )
                # merge the NCH chunk partials for this batch tile
                gneg = smlp.tile([128, 1], F32, tag="gneg")  # -global max
                nc.vector.tensor_reduce(
                    out=gneg[:],
                    in_=nmx[:, bt * NCH : (bt + 1) * NCH],
                    axis=mybir.AxisListType.X,
                    op=mybir.AluOpType.min,
                )
                e = smlp.tile([128, NCH], F32, tag="e")  # exp(chunk_max - M)
                nc.scalar.activation(
                    out=e[:],
                    in_=nmx[:, bt * NCH : (bt + 1) * NCH],
                    func=mybir.ActivationFunctionType.Exp,
                    bias=gneg[:],
                    scale=-1.0,
                )
                se = smlp.tile([128, NCH], F32, tag="se")
                nc.vector.tensor_mul(
                    out=se[:], in0=e[:], in1=ssm[:, bt * NCH : (bt + 1) * NCH]
                )
                nc.vector.reduce_sum(
                    out=sm_sb[:, bt : bt + 1], in_=se[:], axis=mybir.AxisListType.X
                )
                nc.scalar.mul(out=mx_sb[:, bt : bt + 1], in_=gneg[:], mul=-1.0)
            nc.sync.dma_start(out=mx[:], in_=mx_sb[:])
            nc.sync.dma_start(out=sm[:], in_=sm_sb[:])

    _legalize_waits(nc)
    return nc


def _get_nc(nslots) -> bass.Bass:
    key = tuple(nslots)
    if key not in _NC_CACHE:
        _NC_CACHE[key] = build_nc(nslots)
    return _NC_CACHE[key]


def _prep_inputs(weights, x, path_idx, path_len):
    """Host-side input staging (layout/indices only, no FLOPs on payload).

    Per core: sort classes by path_len descending so tile t only needs
    1 + max(len in tile) gather slots. Returns (in_maps, nslots, invs)
    where invs[c] unpermutes that core's class order.
    """
    w_ext = np.empty((LEXT, D), np.float32)
    w_ext[:LENGTH] = np.asarray(weights, np.float32)
    w_ext[LENGTH] = 0.0
    xt = np.ascontiguousarray(np.asarray(x, np.float32).T)  # [D, B]

    pidx = np.asarray(path_idx, np.int64)
    plen = np.asarray(path_len, np.int64)
    lens_all = np.clip(plen, 0, DMAX)
    # global sort by path length, dealt round-robin to cores: every core's
    # position-j class has (nearly) the same length rank, so the shared
    # per-tile slot counts are minimal.
    order_g = np.argsort(-lens_all, kind="stable")  # [K]
    in_maps, clss = [], []
    nslots_cores = np.zeros((NCORES, NT), np.int64)
    for c in range(NCORES):
        cls = order_g[c::NCORES]  # this core's classes, len-sorted desc
        clss.append(cls)
        lens_s = lens_all[cls]
        idx_ext = np.full((KPAD, S), ZROW, np.int32)
        idx_ext[:KSH, 0] = cls.astype(np.int32)
        slot = np.where(
            np.arange(DMAX)[None, :] < lens_s[:, None],
            pidx[cls],
            ZROW,
        ).astype(np.int32)
        idx_ext[:KSH, 1:] = slot
        lens_pad = np.zeros(KPAD, np.int64)
        lens_pad[:KSH] = lens_s
        nslots_cores[c] = 1 + lens_pad.reshape(NT, 128).max(axis=1)
        # [KPAD, S] -> [128, NT*S] with idxT[p, t*S+s] = idx_ext[t*128+p, s]
        idxT = np.ascontiguousarray(
            idx_ext.reshape(NT, 128, S).transpose(1, 0, 2).reshape(128, NT * S)
        )
        in_maps.append({"w": w_ext, "xt": xt, "idxT": idxT})
    nslots = [int(v) for v in nslots_cores.max(axis=0)]
    return in_maps, nslots, clss


def _assemble(results, y, clss):
    logits = np.empty((B, K), np.float32)
    added_weights = np.empty((K, D), np.float32)
    for c, r in enumerate(results):
        logits[:, clss[c]] = r["logits_sh"]
        added_weights[clss[c]] = r["aw_sh"]
    # per-core logsumexp partials: mx/sm are [128, NBT] with b = bt*128 + p
    M_c = np.stack([r["mx"].T.reshape(B) for r in results])  # [NCORES, B]
    S_c = np.stack([r["sm"].T.reshape(B) for r in results])  # [NCORES, B]
    M = M_c.max(axis=0)
    S = (S_c * np.exp(M_c - M[None, :])).sum(axis=0)
    lse = M + np.log(S)
    yy = np.asarray(y).astype(np.int64)
    loss = np.float32(np.mean(lse - logits[np.arange(B), yy]))
    return loss, logits, added_weights


def run(inputs: dict, trace: bool = False, trace_cores=None):
    """Run the device kernel; returns ((loss, logits, added_weights), BassKernelResults)."""
    in_maps, nslots, invs = _prep_inputs(
        inputs["weights"], inputs["x"], inputs["path_idx"], inputs["path_len"]
    )
    nc = _get_nc(nslots)
    res = run_bass_kernel_spmd(
        nc, in_maps, core_ids=list(range(NCORES)), trace=trace,
        trace_cores=trace_cores,
    )
    return _assemble(res.results, inputs["y"], invs), res


def kernel(weights, x, y, path_idx, path_len):
    out, _ = run(
        {
            "weights": weights,
            "x": x,
            "y": y,
            "path_idx": path_idx,
            "path_len": path_len,
        }
    )
    return out


# revision 22
# speedup vs baseline: 1.1327x; 1.1327x over previous
"""CoverTreeLoss (hierarchical softmax CE) Trainium2 kernel.

Vocab-parallel over 8 NeuronCores: each core owns K/8 = 6250 classes.
Per core:
  phase A: indirect-DMA gather of each class's path rows (self row + up to
           16 ancestors, padded slots point at an appended zero row) from the
           full weights table in HBM, tree-reduce the 17 slots on DVE,
           transpose on the PE into awT [d=128, classes].
  phase B: logits chunk matmuls x_btT.T @ awT -> PSUM, copy to SBUF, DMA the
           logits shard out; per-chunk neg-max (DVE) + exp-sum (ACT accum)
           produce per-batch-row logsumexp partials.
Host combines the 8 shards: concat logits/added_weights, 8-way logsumexp
merge, mean CE loss.

Self-contained: hardcodes all shapes from the problem spec.
"""

import numpy as np

import concourse.bass as bass
import concourse.mybir as mybir
from concourse.bass_utils import run_bass_kernel_spmd
from concourse.masks import make_identity
from concourse.tile import TileContext

F32 = mybir.dt.float32
I32 = mybir.dt.int32

# Problem shapes (hardcoded per contract).
def configure(k=50000, length=65536, d=128, b=1024, dmax=16, ncores=8):
    global K, LENGTH, D, B, DMAX, NCORES, S, KSH, NT, KPAD, LEXT, ZROW, NBT, CH, NCH
    K, LENGTH, D, B, DMAX, NCORES = k, length, d, b, dmax, ncores
    S = DMAX + 1       # gather slots per class: self row + 16 path slots
    KSH = K // NCORES  # classes per core
    NT = (KSH + 127) // 128          # class tiles per core
    KPAD = NT * 128
    LEXT = LENGTH + 1                # weights + appended zero row
    ZROW = LENGTH                    # index of the zero row
    NBT = B // 128                   # batch tiles
    CH = 512                         # logits chunk (one PSUM bank of fp32)
    NCH = (KPAD + CH - 1) // CH
    _NC_CACHE.clear()


_NC_CACHE: dict = {}
configure()


# Instruction types whose walrus lowering accepts multiple sync waits.
# Compute ISA structs (matmul/TT/activation/...) carry exactly ONE sync-wait
# slot; extra waits must be hoisted onto preceding same-engine NoOps.
_MULTIWAIT_OK: set = set()


def _legalize_waits(nc: bass.Bass) -> None:
    """Compute/SWDGE ISA instructions carry exactly one sync-wait slot.

    Hoist all-but-one wait of any multi-wait instruction onto freshly
    created same-engine NoOps placed immediately before it. NoOps are
    created via the engine API (so CoreSim/bass bookkeeping sees them)
    and then moved into position.
    """
    plan = []  # (fn, blk, inst) with >1 waits
    for fn in nc.m.functions:
        for blk in fn.blocks:
            for inst in blk.instructions:
                si = inst.sync_info
                waits = list(si.on_wait) if si is not None and si.on_wait else []
                if len(waits) > 1 and type(inst).__name__ not in _MULTIWAIT_OK:
                    plan.append((blk, inst))
    for blk, inst in plan:
        si = inst.sync_info
        waits = list(si.on_wait)
        nops = []
        for wx in waits[:-1]:
            bn = nc.engines[inst.engine].nop()
            n = bn.ins
            n.sync_info = mybir.SyncInfo(on_wait=[wx], on_update=[])
            nops.append(n)
        inst.sync_info = mybir.SyncInfo(
            on_wait=[waits[-1]], on_update=list(si.on_update or [])
        )
        # remove the freshly appended nops from wherever add_instruction put
        # them, then splice before the target instruction
        nop_names = {n.name for n in nops}
        for fn2 in nc.m.functions:
            for b2 in fn2.blocks:
                if any(i.name in nop_names for i in b2.instructions):
                    b2.instructions = [
                        i for i in b2.instructions if i.name not in nop_names
                    ]
        insts = list(blk.instructions)
        pos = next(k for k, i in enumerate(insts) if i.name == inst.name)
        blk.instructions = insts[:pos] + nops + insts[pos:]


def build_nc(nslots) -> bass.Bass:
    nc = bass.Bass(target_bir_lowering=False)

    w = nc.dram_tensor("w", [LEXT, D], F32, kind="ExternalInput")
    xt = nc.dram_tensor("xt", [D, B], F32, kind="ExternalInput")
    idxT = nc.dram_tensor("idxT", [D, NT * S], I32, kind="ExternalInput")

    logits_sh = nc.dram_tensor("logits_sh", [B, KSH], F32, kind="ExternalOutput")
    aw_sh = nc.dram_tensor("aw_sh", [KSH, D], F32, kind="ExternalOutput")
    mx = nc.dram_tensor("mx", [128, NBT], F32, kind="ExternalOutput")
    sm = nc.dram_tensor("sm", [128, NBT], F32, kind="ExternalOutput")

    with TileContext(nc) as tc:
        with (
            tc.tile_pool(name="const", bufs=1) as constp,
            tc.tile_pool(name="gath", bufs=3) as gathp,
            tc.tile_pool(name="lout", bufs=3) as loutp,
            tc.tile_pool(name="scr", bufs=2) as scrp,
            tc.tile_pool(name="sml", bufs=4) as smlp,
            tc.tile_pool(name="pst", bufs=2, space="PSUM") as psumt,
            tc.tile_pool(name="psm", bufs=3, space="PSUM") as psummm,
        ):
            ident = constp.tile([128, 128], F32)
            make_identity(nc, ident[:])

            xt_sb = constp.tile([D, B], F32)
            nc.sync.dma_start(out=xt_sb[:], in_=xt[:])
            idx_sb = constp.tile([D, NT * S], I32)
            nc.sync.dma_start(out=idx_sb[:], in_=idxT[:])

            awT = constp.tile([D, KPAD], F32)       # [d, class] transposed
            nmx = constp.tile([128, NBT * NCH], F32)  # per-chunk -max
            ssm = constp.tile([128, NBT * NCH], F32)  # per-chunk sum(exp(.-max))
            mx_sb = constp.tile([128, NBT], F32)
            sm_sb = constp.tile([128, NBT], F32)

            # ---------------- Phase A: gather + segment-sum + transpose ----
            # One indirect DMA gathers 128 rows (one per partition). Classes
            # are pre-sorted by path length (host) so tile t only needs
            # nslots[t] = 1 + max path_len in the tile gather instructions.
            for t in range(NT):
                n = nslots[t]
                g = gathp.tile([128, S, D], F32, tag="g")
                for s in range(n):
                    nc.gpsimd.indirect_dma_start(
                        out=g[:, s, :],
                        out_offset=None,
                        in_=w[:],
                        in_offset=bass.IndirectOffsetOnAxis(
                            ap=idx_sb[:, t * S + s : t * S + s + 1], axis=0
                        ),
                    )
                # fold upper half into lower half until one slot remains
                m = n
                while m > 1:
                    h = m // 2
                    nc.vector.tensor_add(
                        out=g[:, 0:h, :],
                        in0=g[:, 0:h, :],
                        in1=g[:, m - h : m, :],
                    )
                    m -= h
                rows = min(128, KSH - t * 128)
                nc.sync.dma_start(
                    out=aw_sh[t * 128 : t * 128 + rows, :], in_=g[:rows, 0, :]
                )
                pt = psumt.tile([128, 128], F32, tag="pt")
                nc.tensor.transpose(out=pt[:], in_=g[:, 0, :], identity=ident[:])
                nc.scalar.copy(out=awT[:, t * 128 : (t + 1) * 128], in_=pt[:])

            # ---------------- Phase B: logits + softmax partials -----------
            # chunk-outer order: chunk c only needs awT class tiles
            # 4c..4c+3, so B streams while phase A is still gathering.
            for c in range(NCH):
                for bt in range(NBT):
                    c0 = c * CH
                    wf = min(CH, KPAD - c0)          # cols incl. padding
                    wv = min(KSH - c0, wf)           # valid (real-class) cols
                    pm = psummm.tile([128, CH], F32, tag="pm")
                    nc.tensor.matmul(
                        out=pm[:, :wf],
                        lhsT=xt_sb[:, bt * 128 : (bt + 1) * 128],
                        rhs=awT[:, c0 : c0 + wf],
                        start=True,
                        stop=True,
                    )
                    lc = loutp.tile([128, CH], F32, tag="lc")
                    nc.scalar.copy(out=lc[:, :wv], in_=pm[:, :wv])
                    i1 = bt * NCH + c
                    nc.vector.reduce_max(
                        out=nmx[:, i1 : i1 + 1],
                        in_=lc[:, :wv],
                        axis=mybir.AxisListType.X,
                        negate=True,
                    )
                    sc = scrp.tile([128, CH], F32, tag="sc")
                    nc.scalar.activation(
                        out=sc[:, :wv],
                        in_=lc[:, :wv],
                        func=mybir.ActivationFunctionType.Exp,
                        bias=nmx[:, i1 : i1 + 1],
                        scale=1.0,
                        accum_out=ssm[:, i1 : i1 + 1],
                    )
                    nc.sync.dma_start(
                        out=logits_sh[bt * 128 : (bt + 1) * 128, c0 : c0 + wv],
                        in_=lc[:, :wv],
                    )
            for bt in range(NBT):
                # merge the NCH chunk partials for this batch tile
                gneg = smlp.tile([128, 1], F32, tag="gneg")  # -global max
                nc.vector.tensor_reduce(
                    out=gneg[:],
                    in_=nmx[:, bt * NCH : (bt + 1) * NCH],
                    axis=mybir.AxisListType.X,
                    op=mybir.AluOpType.min,
                )
                e = smlp.tile([128, NCH], F32, tag="e")  # exp(chunk_max - M)
                nc.scalar.activation(
                    out=e[:],
                    in_=nmx[:, bt * NCH : (bt + 1) * NCH],
                    func=mybir.ActivationFunctionType.Exp,
                    bias=gneg[:],
                    scale=-1.0,
                )
                se = smlp.tile([128, NCH], F32, tag="se")
                nc.vector.tensor_mul(
                    out=se[:], in0=e[:], in1=ssm[:, bt * NCH : (bt + 1) * NCH]
                )
                nc.vector.reduce_sum(
                    out=sm_sb[:, bt : bt + 1], in_=se[:], axis=mybir.AxisListType.X
                )
                nc.scalar.mul(out=mx_sb[:, bt : bt + 1], in_=gneg[:], mul=-1.0)
            nc.sync.dma_start(out=mx[:], in_=mx_sb[:])
            nc.sync.dma_start(out=sm[:], in_=sm_sb[:])

    _legalize_waits(nc)
    return nc


def _get_nc(nslots) -> bass.Bass:
    key = tuple(nslots)
    if key not in _NC_CACHE:
        _NC_CACHE[key] = build_nc(nslots)
    return _NC_CACHE[key]


def _prep_inputs(weights, x, path_idx, path_len):
    """Host-side input staging (layout/indices only, no FLOPs on payload).

    Per core: sort classes by path_len descending so tile t only needs
    1 + max(len in tile) gather slots. Returns (in_maps, nslots, invs)
    where invs[c] unpermutes that core's class order.
    """
    w_ext = np.empty((LEXT, D), np.float32)
    w_ext[:LENGTH] = np.asarray(weights, np.float32)
    w_ext[LENGTH] = 0.0
    xt = np.ascontiguousarray(np.asarray(x, np.float32).T)  # [D, B]

    pidx = np.asarray(path_idx, np.int64)
    plen = np.asarray(path_len, np.int64)
    lens_all = np.clip(plen, 0, DMAX)
    # global sort by path length, dealt round-robin to cores: every core's
    # position-j class has (nearly) the same length rank, so the shared
    # per-tile slot counts are minimal.
    order_g = np.argsort(-lens_all, kind="stable")  # [K]
    in_maps, clss = [], []
    nslots_cores = np.zeros((NCORES, NT), np.int64)
    for c in range(NCORES):
        cls = order_g[c::NCORES]  # this core's classes, len-sorted desc
        clss.append(cls)
        lens_s = lens_all[cls]
        idx_ext = np.full((KPAD, S), ZROW, np.int32)
        idx_ext[:KSH, 0] = cls.astype(np.int32)
        slot = np.where(
            np.arange(DMAX)[None, :] < lens_s[:, None],
            pidx[cls],
            ZROW,
        ).astype(np.int32)
        idx_ext[:KSH, 1:] = slot
        lens_pad = np.zeros(KPAD, np.int64)
        lens_pad[:KSH] = lens_s
        nslots_cores[c] = 1 + lens_pad.reshape(NT, 128).max(axis=1)
        # [KPAD, S] -> [128, NT*S] with idxT[p, t*S+s] = idx_ext[t*128+p, s]
        idxT = np.ascontiguousarray(
            idx_ext.reshape(NT, 128, S).transpose(1, 0, 2).reshape(128, NT * S)
        )
        in_maps.append({"w": w_ext, "xt": xt, "idxT": idxT})
    nslots = [int(v) for v in nslots_cores.max(axis=0)]
    return in_maps, nslots, clss


def _assemble(results, y, clss):
    logits = np.empty((B, K), np.float32)
    added_weights = np.empty((K, D), np.float32)
    for c, r in enumerate(results):
        logits[:, clss[c]] = r["logits_sh"]
        added_weights[clss[c]] = r["aw_sh"]
    # per-core logsumexp partials: mx/sm are [128, NBT] with b = bt*128 + p
    M_c = np.stack([r["mx"].T.reshape(B) for r in results])  # [NCORES, B]
    S_c = np.stack([r["sm"].T.reshape(B) for r in results])  # [NCORES, B]
    M = M_c.max(axis=0)
    S = (S_c * np.exp(M_c - M[None, :])).sum(axis=0)
    lse = M + np.log(S)
    yy = np.asarray(y).astype(np.int64)
    loss = np.float32(np.mean(lse - logits[np.arange(B), yy]))
    return loss, logits, added_weights


def run(inputs: dict, trace: bool = False, trace_cores=None):
    """Run the device kernel; returns ((loss, logits, added_weights), BassKernelResults)."""
    in_maps, nslots, invs = _prep_inputs(
        inputs["weights"], inputs["x"], inputs["path_idx"], inputs["path_len"]
    )
    nc = _get_nc(nslots)
    res = run_bass_kernel_spmd(
        nc, in_maps, core_ids=list(range(NCORES)), trace=trace,
        trace_cores=trace_cores,
    )
    return _assemble(res.results, inputs["y"], invs), res


def kernel(weights, x, y, path_idx, path_len):
    out, _ = run(
        {
            "weights": weights,
            "x": x,
            "y": y,
            "path_idx": path_idx,
            "path_len": path_len,
        }
    )
    return out


# revision 24
# speedup vs baseline: 1.1339x; 1.0011x over previous
"""CoverTreeLoss (hierarchical softmax CE) Trainium2 kernel.

Vocab-parallel over 8 NeuronCores: each core owns K/8 = 6250 classes.
Per core:
  phase A: indirect-DMA gather of each class's path rows (self row + up to
           16 ancestors, padded slots point at an appended zero row) from the
           full weights table in HBM, tree-reduce the 17 slots on DVE,
           transpose on the PE into awT [d=128, classes].
  phase B: logits chunk matmuls x_btT.T @ awT -> PSUM, copy to SBUF, DMA the
           logits shard out; per-chunk neg-max (DVE) + exp-sum (ACT accum)
           produce per-batch-row logsumexp partials.
Host combines the 8 shards: concat logits/added_weights, 8-way logsumexp
merge, mean CE loss.

Self-contained: hardcodes all shapes from the problem spec.
"""

import numpy as np

import concourse.bass as bass
import concourse.mybir as mybir
from concourse.bass_utils import run_bass_kernel_spmd
from concourse.masks import make_identity
from concourse.tile import TileContext

F32 = mybir.dt.float32
I32 = mybir.dt.int32

# Problem shapes (hardcoded per contract).
def configure(k=50000, length=65536, d=128, b=1024, dmax=16, ncores=8):
    global K, LENGTH, D, B, DMAX, NCORES, S, KSH, NT, KPAD, LEXT, ZROW, NBT, CH, NCH
    K, LENGTH, D, B, DMAX, NCORES = k, length, d, b, dmax, ncores
    S = DMAX + 1       # gather slots per class: self row + 16 path slots
    KSH = K // NCORES  # classes per core
    NT = (KSH + 127) // 128          # class tiles per core
    KPAD = NT * 128
    LEXT = LENGTH + 1                # weights + appended zero row
    ZROW = LENGTH                    # index of the zero row
    NBT = B // 128                   # batch tiles
    CH = 512                         # logits chunk (one PSUM bank of fp32)
    NCH = (KPAD + CH - 1) // CH
    _NC_CACHE.clear()


_NC_CACHE: dict = {}
configure()


# Instruction types whose walrus lowering accepts multiple sync waits.
# Compute ISA structs (matmul/TT/activation/...) carry exactly ONE sync-wait
# slot; extra waits must be hoisted onto preceding same-engine NoOps.
_MULTIWAIT_OK: set = set()


def _legalize_waits(nc: bass.Bass) -> None:
    """Compute/SWDGE ISA instructions carry exactly one sync-wait slot.

    Hoist all-but-one wait of any multi-wait instruction onto freshly
    created same-engine NoOps placed immediately before it. NoOps are
    created via the engine API (so CoreSim/bass bookkeeping sees them)
    and then moved into position.
    """
    plan = []  # (fn, blk, inst) with >1 waits
    for fn in nc.m.functions:
        for blk in fn.blocks:
            for inst in blk.instructions:
                si = inst.sync_info
                waits = list(si.on_wait) if si is not None and si.on_wait else []
                if len(waits) > 1 and type(inst).__name__ not in _MULTIWAIT_OK:
                    plan.append((blk, inst))
    for blk, inst in plan:
        si = inst.sync_info
        waits = list(si.on_wait)
        nops = []
        for wx in waits[:-1]:
            bn = nc.engines[inst.engine].nop()
            n = bn.ins
            n.sync_info = mybir.SyncInfo(on_wait=[wx], on_update=[])
            nops.append(n)
        inst.sync_info = mybir.SyncInfo(
            on_wait=[waits[-1]], on_update=list(si.on_update or [])
        )
        # remove the freshly appended nops from wherever add_instruction put
        # them, then splice before the target instruction
        nop_names = {n.name for n in nops}
        for fn2 in nc.m.functions:
            for b2 in fn2.blocks:
                if any(i.name in nop_names for i in b2.instructions):
                    b2.instructions = [
                        i for i in b2.instructions if i.name not in nop_names
                    ]
        insts = list(blk.instructions)
        pos = next(k for k, i in enumerate(insts) if i.name == inst.name)
        blk.instructions = insts[:pos] + nops + insts[pos:]


def build_nc(nslots) -> bass.Bass:
    nc = bass.Bass(target_bir_lowering=False)

    w = nc.dram_tensor("w", [LEXT, D], F32, kind="ExternalInput")
    xt = nc.dram_tensor("xt", [D, B], F32, kind="ExternalInput")
    idxT = nc.dram_tensor("idxT", [D, NT * S], I32, kind="ExternalInput")

    logits_sh = nc.dram_tensor("logits_sh", [B, KSH], F32, kind="ExternalOutput")
    aw_sh = nc.dram_tensor("aw_sh", [KSH, D], F32, kind="ExternalOutput")
    mx = nc.dram_tensor("mx", [128, NBT], F32, kind="ExternalOutput")
    sm = nc.dram_tensor("sm", [128, NBT], F32, kind="ExternalOutput")

    with TileContext(nc) as tc:
        with (
            tc.tile_pool(name="const", bufs=1) as constp,
            tc.tile_pool(name="gath", bufs=6) as gathp,
            tc.tile_pool(name="lout", bufs=3) as loutp,
            tc.tile_pool(name="scr", bufs=2) as scrp,
            tc.tile_pool(name="sml", bufs=4) as smlp,
            tc.tile_pool(name="pst", bufs=2, space="PSUM") as psumt,
            tc.tile_pool(name="psm", bufs=3, space="PSUM") as psummm,
        ):
            ident = constp.tile([128, 128], F32)
            make_identity(nc, ident[:])

            xt_sb = constp.tile([D, B], F32)
            nc.sync.dma_start(out=xt_sb[:], in_=xt[:])
            idx_sb = constp.tile([D, NT * S], I32)
            # split the index load so tile 0's gathers start immediately
            nc.sync.dma_start(out=idx_sb[:, :S], in_=idxT[:, :S])
            nc.sync.dma_start(out=idx_sb[:, S:], in_=idxT[:, S:])

            awT = constp.tile([D, KPAD], F32)       # [d, class] transposed
            nmx = constp.tile([128, NBT * NCH], F32)  # per-chunk -max
            ssm = constp.tile([128, NBT * NCH], F32)  # per-chunk sum(exp(.-max))
            mx_sb = constp.tile([128, NBT], F32)
            sm_sb = constp.tile([128, NBT], F32)

            # ---------------- Phase A: gather + segment-sum + transpose ----
            # One indirect DMA gathers 128 rows (one per partition). Classes
            # are pre-sorted by path length (host) so tile t only needs
            # nslots[t] = 1 + max path_len in the tile gather instructions.
            for t in range(NT):
                n = nslots[t]
                g = gathp.tile([128, S, D], F32, tag="g")
                for s in range(n):
                    nc.gpsimd.indirect_dma_start(
                        out=g[:, s, :],
                        out_offset=None,
                        in_=w[:],
                        in_offset=bass.IndirectOffsetOnAxis(
                            ap=idx_sb[:, t * S + s : t * S + s + 1], axis=0
                        ),
                    )
                # fold upper half into lower half until one slot remains
                m = n
                while m > 1:
                    h = m // 2
                    nc.vector.tensor_add(
                        out=g[:, 0:h, :],
                        in0=g[:, 0:h, :],
                        in1=g[:, m - h : m, :],
                    )
                    m -= h
                rows = min(128, KSH - t * 128)
                nc.sync.dma_start(
                    out=aw_sh[t * 128 : t * 128 + rows, :], in_=g[:rows, 0, :]
                )
                pt = psumt.tile([128, 128], F32, tag="pt")
                nc.tensor.transpose(out=pt[:], in_=g[:, 0, :], identity=ident[:])
                nc.scalar.copy(out=awT[:, t * 128 : (t + 1) * 128], in_=pt[:])

            # ---------------- Phase B: logits + softmax partials -----------
            # chunk-outer order: chunk c only needs awT class tiles
            # 4c..4c+3, so B streams while phase A is still gathering.
            for c in range(NCH):
                for bt in range(NBT):
                    c0 = c * CH
                    wf = min(CH, KPAD - c0)          # cols incl. padding
                    wv = min(KSH - c0, wf)           # valid (real-class) cols
                    pm = psummm.tile([128, CH], F32, tag="pm")
                    nc.tensor.matmul(
                        out=pm[:, :wf],
                        lhsT=xt_sb[:, bt * 128 : (bt + 1) * 128],
                        rhs=awT[:, c0 : c0 + wf],
                        start=True,
                        stop=True,
                    )
                    lc = loutp.tile([128, CH], F32, tag="lc")
                    nc.scalar.copy(out=lc[:, :wv], in_=pm[:, :wv])
                    i1 = bt * NCH + c
                    nc.vector.reduce_max(
                        out=nmx[:, i1 : i1 + 1],
                        in_=lc[:, :wv],
                        axis=mybir.AxisListType.X,
                        negate=True,
                    )
                    sc = scrp.tile([128, CH], F32, tag="sc")
                    nc.scalar.activation(
                        out=sc[:, :wv],
                        in_=lc[:, :wv],
                        func=mybir.ActivationFunctionType.Exp,
                        bias=nmx[:, i1 : i1 + 1],
                        scale=1.0,
                        accum_out=ssm[:, i1 : i1 + 1],
                    )
                    nc.sync.dma_start(
                        out=logits_sh[bt * 128 : (bt + 1) * 128, c0 : c0 + wv],
                        in_=lc[:, :wv],
                    )
            for bt in range(NBT):
                # merge the NCH chunk partials for this batch tile
                gneg = smlp.tile([128, 1], F32, tag="gneg")  # -global max
                nc.vector.tensor_reduce(
                    out=gneg[:],
                    in_=nmx[:, bt * NCH : (bt + 1) * NCH],
                    axis=mybir.AxisListType.X,
                    op=mybir.AluOpType.min,
                )
                e = smlp.tile([128, NCH], F32, tag="e")  # exp(chunk_max - M)
                nc.scalar.activation(
                    out=e[:],
                    in_=nmx[:, bt * NCH : (bt + 1) * NCH],
                    func=mybir.ActivationFunctionType.Exp,
                    bias=gneg[:],
                    scale=-1.0,
                )
                se = smlp.tile([128, NCH], F32, tag="se")
                nc.vector.tensor_mul(
                    out=se[:], in0=e[:], in1=ssm[:, bt * NCH : (bt + 1) * NCH]
                )
                nc.vector.reduce_sum(
                    out=sm_sb[:, bt : bt + 1], in_=se[:], axis=mybir.AxisListType.X
                )
                nc.scalar.mul(out=mx_sb[:, bt : bt + 1], in_=gneg[:], mul=-1.0)
            nc.sync.dma_start(out=mx[:], in_=mx_sb[:])
            nc.sync.dma_start(out=sm[:], in_=sm_sb[:])

    _legalize_waits(nc)
    return nc


def _get_nc(nslots) -> bass.Bass:
    key = tuple(nslots)
    if key not in _NC_CACHE:
        _NC_CACHE[key] = build_nc(nslots)
    return _NC_CACHE[key]


def _prep_inputs(weights, x, path_idx, path_len):
    """Host-side input staging (layout/indices only, no FLOPs on payload).

    Per core: sort classes by path_len descending so tile t only needs
    1 + max(len in tile) gather slots. Returns (in_maps, nslots, invs)
    where invs[c] unpermutes that core's class order.
    """
    w_ext = np.empty((LEXT, D), np.float32)
    w_ext[:LENGTH] = np.asarray(weights, np.float32)
    w_ext[LENGTH] = 0.0
    xt = np.ascontiguousarray(np.asarray(x, np.float32).T)  # [D, B]

    pidx = np.asarray(path_idx, np.int64)
    plen = np.asarray(path_len, np.int64)
    lens_all = np.clip(plen, 0, DMAX)
    # global sort by path length, dealt round-robin to cores: every core's
    # position-j class has (nearly) the same length rank, so the shared
    # per-tile slot counts are minimal.
    order_g = np.argsort(-lens_all, kind="stable")  # [K]
    in_maps, clss = [], []
    nslots_cores = np.zeros((NCORES, NT), np.int64)
    for c in range(NCORES):
        cls = order_g[c::NCORES]  # this core's classes, len-sorted desc
        clss.append(cls)
        lens_s = lens_all[cls]
        idx_ext = np.full((KPAD, S), ZROW, np.int32)
        idx_ext[:KSH, 0] = cls.astype(np.int32)
        slot = np.where(
            np.arange(DMAX)[None, :] < lens_s[:, None],
            pidx[cls],
            ZROW,
        ).astype(np.int32)
        idx_ext[:KSH, 1:] = slot
        lens_pad = np.zeros(KPAD, np.int64)
        lens_pad[:KSH] = lens_s
        nslots_cores[c] = 1 + lens_pad.reshape(NT, 128).max(axis=1)
        # [KPAD, S] -> [128, NT*S] with idxT[p, t*S+s] = idx_ext[t*128+p, s]
        idxT = np.ascontiguousarray(
            idx_ext.reshape(NT, 128, S).transpose(1, 0, 2).reshape(128, NT * S)
        )
        in_maps.append({"w": w_ext, "xt": xt, "idxT": idxT})
    nslots = [int(v) for v in nslots_cores.max(axis=0)]
    return in_maps, nslots, clss


def _assemble(results, y, clss):
    logits = np.empty((B, K), np.float32)
    added_weights = np.empty((K, D), np.float32)
    for c, r in enumerate(results):
        logits[:, clss[c]] = r["logits_sh"]
        added_weights[clss[c]] = r["aw_sh"]
    # per-core logsumexp partials: mx/sm are [128, NBT] with b = bt*128 + p
    M_c = np.stack([r["mx"].T.reshape(B) for r in results])  # [NCORES, B]
    S_c = np.stack([r["sm"].T.reshape(B) for r in results])  # [NCORES, B]
    M = M_c.max(axis=0)
    S = (S_c * np.exp(M_c - M[None, :])).sum(axis=0)
    lse = M + np.log(S)
    yy = np.asarray(y).astype(np.int64)
    loss = np.float32(np.mean(lse - logits[np.arange(B), yy]))
    return loss, logits, added_weights


def run(inputs: dict, trace: bool = False, trace_cores=None):
    """Run the device kernel; returns ((loss, logits, added_weights), BassKernelResults)."""
    in_maps, nslots, invs = _prep_inputs(
        inputs["weights"], inputs["x"], inputs["path_idx"], inputs["path_len"]
    )
    nc = _get_nc(nslots)
    res = run_bass_kernel_spmd(
        nc, in_maps, core_ids=list(range(NCORES)), trace=trace,
        trace_cores=trace_cores,
    )
    return _assemble(res.results, inputs["y"], invs), res


def kernel(weights, x, y, path_idx, path_len):
    out, _ = run(
        {
            "weights": weights,
            "x": x,
            "y": y,
            "path_idx": path_idx,
            "path_len": path_len,
        }
    )
    return out
